# revision 1
# baseline (speedup 1.0000x reference)
"""Trainium2 Bass kernel for nn_ECA_69544110457542.

Math (per row r=(b,t)):
  dyn   = x[:, :31] @ Wd + bd
  value = x[:, 31] * Wv[0] + bv
  xhn   = [dyn | human@Wh+bh | nature@Wn+bn]                      (768 ch)
  pre_j = sum_k cw[t,k] * xhn[perm[ainv[j]+k-3]] + conv_b[t]      (j<256)
  sel   = softmax(relu(pre))
  out   = 0.5*(dyn*sel) @ Wvd1  +  0.5*dyn @ Wvd1 + value @ Wvd2 + bvd
          `------ device ------'  `------- folded into Wfold (host) -----'

Key folds / layout choices:
  - channel shuffle + depthwise-conv gather -> permuted weight matrix Wg
    [193, 1280] whose column (k*256+j) reproduces xhn[:, perm[ainv[j]+k-3]];
  - all purely-linear output terms -> Wfold (one K=33 fp32 matmul);
  - activations stored PRE-TRANSPOSED in DRAM (bf16 for the softmax path,
    fp32 x for the linear path) so lhsT tiles load straight off DMA;
  - the whole softmax path (a <= few % correction of the output) runs in
    bf16: G matmuls, conv combine, exp, gate, z @ Wvd1;
  - per-t conv weights cw[t,k] are applied FREE inside the ACT-engine PSUM
    drains (activation scale= is a per-partition AP), so the DVE combine is
    just 4 bf16 adds;
  - exp's accum_out gives the softmax denominator for free;
  - the 0.5 gate factor is folded into Wvd1.

Sharding: pure data parallel, 32 batches per core on 8 cores.
"""

import sys

sys.path.insert(0, "/opt/trn_rl_repo")

from contextlib import ExitStack

import ml_dtypes
import numpy as np

import concourse.bass as bass
import concourse.tile as tile
from concourse import mybir
from concourse.tile import add_dep_helper
from concourse.bass_utils import run_bass_kernel_spmd

# problem constants
B, T, E = 256, 64, 256
XS, DS = 32, 31
HT, NT_ = 80, 80
C = 3 * E
KW = 5
NCORES = 8
BPC = B // NCORES          # 32 batches per core
R = BPC * T                # 2048 rows per core
P = 128
NTILES = R // P            # 16
AK = XS + 1 + HT + NT_     # 193 act rows: x(32) | ones | h(80) | n(80)
K2 = AK - 128              # 65
NG = KW * E                # 1280 gathered columns
NA = 640                   # G psum half A: slices k0,k1,k2[:128]
NB = 640                   # half B: k2[128:],k3,k4

# packed-constants layout, fp32 slot offsets in [128, WPACK]
O_WG1 = 0                   # bf16 [128, 1280] -> 640 slots
O_WG2 = O_WG1 + NG // 2     # 640:  bf16 [65, 1280] -> 640 slots
O_WDYN = O_WG2 + NG // 2    # 1280: bf16 [33, 256] -> 128 slots
O_WFOLD = O_WDYN + E // 2   # 1408: fp32 [33, 256]
O_WV1 = O_WFOLD + E         # 1664: bf16 0.5*Wvd1 [256, 256] -> 256 slots
O_IDB = O_WV1 + E           # 1920: bf16 identity -> 64 slots
O_CW = O_IDB + P // 2       # 1984: fp32 [128, 5]
O_CB = O_CW + KW            # 1989: fp32 [128, 1]
WPACK = O_CB + 1            # 1990

F32 = mybir.dt.float32
BF16 = mybir.dt.bfloat16
MULT = mybir.AluOpType.mult
ADD = mybir.AluOpType.add
IDENT = mybir.ActivationFunctionType.Identity

_NC_CACHE = None
LAST_RESULTS = None
TRACE = False


def _patched_drain_and_barrier(self, tick_clock, wait_clock):
    # The stock kernel-tail drain puts every processor's final-tick wait on a
    # single Drain instruction; this walrus build rejects multi-wait
    # instructions, so spread the waits over a chain of drains instead.
    import bass_rust as _br
    from concourse.vector_clock import ScopedClock

    nc = self.nc
    drain_inst = nc.sync.drain()
    wait_clock.add_sem_waits(
        drain_inst.ins, ScopedClock({None: tick_clock.global_clock})
    )
    si = drain_inst.ins.sync_info
    if si is not None and len(si.on_wait) > 1:
        waits = list(si.on_wait)
        drain_inst.ins.sync_info = _br.SyncInfo(
            on_wait=[waits[0]], on_update=list(si.on_update)
        )
        for w in waits[1:]:
            d2 = nc.sync.drain()
            d2.ins.sync_info = _br.SyncInfo(on_wait=[w], on_update=[])
    nc.all_engine_barrier()
    assert self.sems is not None
    popped = nc._tile_sem_poison_stack.pop()
    assert popped is self._sem_poison
    nc.clear_and_free_semaphores(list(self.sems.allocated().values()))
    nc.all_engine_barrier()


tile.TileContext._drain_and_barrier = _patched_drain_and_barrier


def _build_nc():
    nc = bass.Bass()
    actb_d = nc.dram_tensor("actb", [P, 2 * R], BF16, kind="ExternalInput")
    actx_d = nc.dram_tensor("actx", [33, R], F32, kind="ExternalInput")
    wpack_d = nc.dram_tensor("wpack", [P, WPACK], F32, kind="ExternalInput")
    out_d = nc.dram_tensor("out", [R, E], F32, kind="ExternalOutput")
    actb3 = actb_d[:, :].rearrange("p (two r) -> p two r", two=2)

    with tile.TileContext(nc) as tc, ExitStack() as ctx:
        consts = ctx.enter_context(tc.tile_pool(name="consts", bufs=1))
        pactB = ctx.enter_context(tc.tile_pool(name="pactB", bufs=16))
        pactX = ctx.enter_context(tc.tile_pool(name="pactX", bufs=16))
        pgs = ctx.enter_context(tc.tile_pool(name="pgs", bufs=3))
        pacc = ctx.enter_context(tc.tile_pool(name="pacc", bufs=8))
        pex = ctx.enter_context(tc.tile_pool(name="pex", bufs=4))
        psml = ctx.enter_context(tc.tile_pool(name="psml", bufs=8))
        pz = ctx.enter_context(tc.tile_pool(name="pz", bufs=4))
        pzT = ctx.enter_context(tc.tile_pool(name="pzT", bufs=4))
        pot = ctx.enter_context(tc.tile_pool(name="pot", bufs=4))
        ptch = ctx.enter_context(tc.tile_pool(name="ptch", bufs=4))
        pG = ctx.enter_context(tc.tile_pool(name="pG", bufs=2, space="PSUM"))
        pdf = ctx.enter_context(tc.tile_pool(name="pdf", bufs=2, space="PSUM"))
        ptz = ctx.enter_context(tc.tile_pool(name="ptz", bufs=2, space="PSUM"))

        wp = consts.tile([P, WPACK], F32)
        nc.sync.dma_start(wp[:], wpack_d[:, :])
        wpb = wp[:].bitcast(BF16)
        wg1 = wpb[:, 2 * O_WG1 : 2 * O_WG1 + NG]
        wg2 = wpb[0:K2, 2 * O_WG2 : 2 * O_WG2 + NG]
        wdyn = wpb[0:33, 2 * O_WDYN : 2 * O_WDYN + E]
        wfold = wp[0:33, O_WFOLD : O_WFOLD + E]
        wv1a = wpb[:, 2 * O_WV1 : 2 * O_WV1 + E]
        wv1b = wpb[:, 2 * O_WV1 + E : 2 * O_WV1 + 2 * E]
        identb = wpb[:, 2 * O_IDB : 2 * O_IDB + P]
        cw = wp[:, O_CW : O_CW + KW]
        cb = wp[:, O_CB : O_CB + 1]

        # PE/DVE observe the weights DMA once (compute instructions carry
        # only ONE sem-wait on this walrus build)
        scr = ptz.tile([P, 1], F32, tag="ptz")
        nc.tensor.matmul(scr[:], identb, identb[:, 0:1], start=True, stop=True)
        wtouch = psml.tile([P, 1], F32, tag="sml")
        nc.vector.tensor_copy(wtouch[:], cb)
        stouch = psml.tile([P, 1], F32, tag="sml")
        nc.scalar.copy(stouch[:], cb)

        z_prev = None
        z_prev2 = None
        gs_prev = None
        mmg_last_prev = None
        mmz_prev = None
        mmz_prev2 = None
        obufs = {}
        pend = None

        def flush_z_impl(pend):
            # z-path of tile j, emitted one iteration later so PE never
            # waits on the current tile's softmax chain
            j, zj, pdfj = pend
            ptz_t = ptz.tile([P, 2, P], BF16, tag="ptz")
            nc.tensor.transpose(ptz_t[:, 0, :], zj[:, 0:128], identb)
            nc.tensor.transpose(ptz_t[:, 1, :], zj[:, 128:256], identb)
            zT = pzT.tile([P, 2, P], BF16)
            nc.vector.tensor_copy(zT[:], ptz_t[:])
            nc.tensor.matmul(
                pdfj[:, 256:512], zT[:, 0, :], wv1a,
                start=False, stop=False, skip_group_check=True,
            )
            mmz2 = nc.tensor.matmul(
                pdfj[:, 256:512], zT[:, 1, :], wv1b,
                start=False, stop=True, skip_group_check=True,
            )
            # ACT observes the gate's DVE tick before the out-copy
            zt_ = ptch.tile([1, 2], BF16, tag="tch2")
            nc.scalar.copy(zt_[:], zj[0:1, 0:2])
            if j % 4 == 0:
                obufs[j // 4] = pot.tile([P, 4, E], F32, tag="obuf", name=f"obuf{j // 4}")
            ob = obufs[j // 4]
            nc.scalar.copy(ob[:, j % 4, :], pdfj[:, 256:512])
            if j % 4 == 3:
                g0 = (j - 3) * P
                odst = out_d[g0 : g0 + 4 * P, :].rearrange(
                    "(t p) e -> p t e", p=P
                )
                nc.gpsimd.dma_start(odst, ob[:])
            return mmz2

        for i in range(NTILES):
            pend_prev = pend
            rows = slice(i * P, (i + 1) * P)
            actb = pactB.tile([P, 2, P], BF16)
            nc.sync.dma_start(actb[:], actb3[:, :, rows])
            actx = pactX.tile([33, P], F32)
            nc.sync.dma_start(actx[:], actx_d[:, rows])

            # "PE observes processor X" gadgets: every real matmul self-loads
            # its weights, so stray LDWEIGHTS are harmless
            absorbers = [
                nc.tensor.ldweights(actb[0:1, 0, 0:2]),
                nc.tensor.ldweights(actx[:].bitcast(BF16)[0:1, 0:2]),
            ]
            if z_prev2 is not None:
                absorbers.append(nc.tensor.ldweights(z_prev2[0:1, 0:2]))
            if gs_prev is not None:
                absorbers.append(nc.tensor.ldweights(gs_prev[0:1, NG - 2 : NG]))
            if mmg_last_prev is not None:
                ldw_self = nc.tensor.ldweights(wpb[0:1, 0:2])
                add_dep_helper(ldw_self.ins, mmg_last_prev.ins, sync=True,
                               reason="absorb PE W-W completion wait")
                if mmz_prev2 is not None:
                    add_dep_helper(ldw_self.ins, mmz_prev2.ins, sync=True,
                                   reason="absorb PE W-W completion wait")
                absorbers.append(ldw_self)

            # dyn (bf16) and folded-linear out part (fp32)
            pdf_t = pdf.tile([P, 512], F32)
            mm_df = nc.tensor.matmul(
                pdf_t[:, 0:E], actb[0:33, 0, :], wdyn, start=True, stop=True
            )
            mm_fo = nc.tensor.matmul(
                pdf_t[:, E:512], actx[:], wfold, start=True, stop=True
            )
            for a in absorbers:
                add_dep_helper(mm_df.ins, a.ins, sync=False,
                               reason="absorbers run before first matmul")
                add_dep_helper(mm_fo.ins, a.ins, sync=False,
                               reason="absorbers run before first matmul")

            # gathered conv operand columns, two psum halves
            pGA = pG.tile([P, NA], F32, tag="G")
            pGB = pG.tile([P, NB], F32, tag="G")
            for gt, c0 in ((pGA, 0), (pGB, NA)):
                for s0, s1 in ((0, 512), (512, 640)):
                    mg = nc.tensor.matmul(
                        gt[:, s0:s1], actb[:, 0, :], wg1[:, c0 + s0 : c0 + s1],
                        start=True, stop=False,
                    )
                    if s0 == 0:
                        for a in absorbers:
                            add_dep_helper(mg.ins, a.ins, sync=False,
                                           reason="absorbers first")
                    mmg_last_prev = nc.tensor.matmul(
                        gt[:, s0:s1], actb[0:K2, 1, :], wg2[:, c0 + s0 : c0 + s1],
                        start=False, stop=True,
                    )


            # ACT observes half A's matmuls, drains A, then B — so drain-A
            # overlaps B's matmuls and the DVE chain starts a drain earlier
            pgtA = ptch.tile([1, 2], BF16, tag="tch2")
            nc.scalar.copy(pgtA[:], pGA[:].bitcast(BF16)[0:1, 0:2])
            gs = pgs.tile([P, NG], BF16)
            nc.scalar.copy(gs[:, 0:NA], pGA[:])
            pgtB = ptch.tile([1, 2], BF16, tag="tch2")
            nc.scalar.copy(pgtB[:], pGB[:].bitcast(BF16)[0:1, 0:2])
            nc.scalar.copy(gs[:, NA:NG], pGB[:])

            # conv combine: scale each k-slice by cw[t,k] (4x-mode
            # tensor_scalar), conv_b folded into the k0 scale op, then add.
            # DVE touches each drain once (single-wait rule) and works on
            # half A while ACT is still draining half B.
            gtA = ptch.tile([1, 2], BF16, tag="tch")
            nc.vector.tensor_copy(gtA[:], gs[0:1, NA - 2 : NA])
            g0 = pacc.tile([P, E], BF16, tag="acc")
            nc.vector.tensor_scalar(g0[:], gs[:, 0:256], cw[:, 0:1], cb,
                                    op0=MULT, op1=ADD)
            g1 = pacc.tile([P, E], BF16, tag="acc")
            nc.vector.tensor_scalar_mul(g1[:], gs[:, 256:512], cw[:, 1:2])
            a01 = pacc.tile([P, E], BF16, tag="acc")
            nc.vector.tensor_add(a01[:], g0[:], g1[:])
            gtB = ptch.tile([1, 2], BF16, tag="tch")
            nc.vector.tensor_copy(gtB[:], gs[0:1, NG - 2 : NG])
            g2 = pacc.tile([P, E], BF16, tag="acc")
            nc.vector.tensor_scalar_mul(g2[:], gs[:, 512:768], cw[:, 2:3])
            g3 = pacc.tile([P, E], BF16, tag="acc")
            nc.vector.tensor_scalar_mul(g3[:], gs[:, 768:1024], cw[:, 3:4])
            g4 = pacc.tile([P, E], BF16, tag="acc")
            nc.vector.tensor_scalar_mul(g4[:], gs[:, 1024:1280], cw[:, 4:5])
            a34 = pacc.tile([P, E], BF16, tag="acc")
            nc.vector.tensor_add(a34[:], g3[:], g4[:])
            a0134 = pacc.tile([P, E], BF16, tag="acc")
            nc.vector.tensor_add(a0134[:], a01[:], a34[:])
            pre = pacc.tile([P, E], BF16, tag="acc")
            nc.vector.tensor_add(pre[:], a0134[:], g2[:])
            relu = pacc.tile([P, E], BF16, tag="acc")
            nc.vector.tensor_scalar_max(relu[:], pre[:], 0.0)

            # exp + free row-sum via accum_out
            exm = pex.tile([P, E], BF16, tag="exm")
            ssum = psml.tile([P, 1], F32, tag="sml")
            nc.scalar.activation(
                exm[:], relu[:], func=mybir.ActivationFunctionType.Exp,
                accum_out=ssum[:],
            )
            sinv = psml.tile([P, 1], F32, tag="sml")
            nc.vector.reciprocal(sinv[:], ssum[:])

            # DVE observes the dyn/fold matmuls once before the gate
            pdtouch = ptch.tile([1, 2], BF16, tag="tch")
            nc.vector.tensor_copy(pdtouch[:], pdf_t[:].bitcast(BF16)[0:1, 0:2])

            # z = (exm / S) * dyn  (the 0.5 is folded into Wvd1)
            z = pz.tile([P, E], BF16, tag="z")
            nc.vector.scalar_tensor_tensor(
                z[:], exm[:], sinv[:], pdf_t[:, 0:E], op0=MULT, op1=MULT
            )
            z_prev2 = z_prev
            z_prev = z
            gs_prev = gs
            pend = (i, z, pdf_t)

            flush_z_impl(pend)

    return nc


def _host_prep(x, human, nature, perm, Wv, bv, Wd, bd, Wh, bh, Wn, bn,
               conv_w, conv_b, Wvd, bvd):
    f = np.float32
    bf = ml_dtypes.bfloat16
    x = np.asarray(x, f)
    human = np.asarray(human, f)
    nature = np.asarray(nature, f)
    Wv = np.asarray(Wv, f); bv = np.asarray(bv, f)
    Wd = np.asarray(Wd, f); bd = np.asarray(bd, f)
    Wh = np.asarray(Wh, f); bh = np.asarray(bh, f)
    Wn = np.asarray(Wn, f); bn = np.asarray(bn, f)
    conv_w = np.asarray(conv_w, f)
    conv_b = np.asarray(conv_b, f)
    Wvd = np.asarray(Wvd, f); bvd = np.asarray(bvd, f)
    perm = np.asarray(perm).astype(np.int64)

    Wvd1 = Wvd[:E, :]
    Wvd2 = Wvd[E:, :]

    acts = np.concatenate(
        [
            x.reshape(B * T, XS),
            np.ones((B * T, 1), f),
            human.reshape(B * T, HT),
            nature.reshape(B * T, NT_),
        ],
        axis=1,
    )
    actsT = np.ascontiguousarray(acts.T)  # [193, B*T]
    actb = np.zeros((P, 2, B * T), bf)
    actb[:, 0, :] = actsT[0:128]
    actb[0:K2, 1, :] = actsT[128:AK]
    actx = np.ascontiguousarray(actsT[0:33])  # fp32 [33, B*T]

    wpack = np.zeros((P, WPACK), f)
    wpv = wpack.view(bf)  # bf16 alias [128, 2*WPACK]

    # folded linear path (fp32)
    wfold = np.zeros((33, E), f)
    wfold[0:DS] = 0.5 * (Wd @ Wvd1)
    wfold[31] = Wv[0] @ Wvd2
    wfold[32] = 0.5 * (bd @ Wvd1) + bv @ Wvd2 + bvd
    wpack[0:33, O_WFOLD : O_WFOLD + E] = wfold

    # dyn (bf16)
    wdyn = np.zeros((33, E), f)
    wdyn[0:DS] = Wd
    wdyn[32] = bd
    wpv[0:33, 2 * O_WDYN : 2 * O_WDYN + E] = wdyn.astype(bf)

    # gathered conv weights (bf16)
    ainv = np.argsort(perm)
    Wg = np.zeros((AK, NG), f)
    for k in range(KW):
        pos = ainv[:E] + k - 3
        for j in range(E):
            pj = pos[j]
            if 0 <= pj < C:
                c = perm[pj]
                col = k * E + j
                if c < E:
                    Wg[0:DS, col] = Wd[:, c]
                    Wg[32, col] = bd[c]
                elif c < 2 * E:
                    Wg[33:113, col] = Wh[:, c - E]
                    Wg[32, col] = bh[c - E]
                else:
                    Wg[113:193, col] = Wn[:, c - 2 * E]
                    Wg[32, col] = bn[c - 2 * E]
    wpv[:, 2 * O_WG1 : 2 * O_WG1 + NG] = Wg[0:128].astype(bf)
    wpv[0:K2, 2 * O_WG2 : 2 * O_WG2 + NG] = Wg[128:AK].astype(bf)

    # 0.5 * Wvd1 (bf16), split into two K-chunks
    wv1 = (0.5 * Wvd1).astype(bf)
    wpv[:, 2 * O_WV1 : 2 * O_WV1 + E] = wv1[0:128]
    wpv[:, 2 * O_WV1 + E : 2 * O_WV1 + 2 * E] = wv1[128:256]

    wpv[:, 2 * O_IDB : 2 * O_IDB + P] = np.eye(P, dtype=bf)
    wpack[:, O_CW : O_CW + KW] = np.tile(conv_w[:, 0, :], (2, 1))
    wpack[:, O_CB] = np.tile(conv_b, 2)
    return actb, actx, wpack


def kernel(**inputs):
    global _NC_CACHE, LAST_RESULTS
    actb, actx, wpack = _host_prep(**inputs)

    if _NC_CACHE is None:
        _NC_CACHE = _build_nc()
    nc = _NC_CACHE

    in_maps = []
    for ci in range(NCORES):
        sb = np.ascontiguousarray(actb[:, :, ci * R : (ci + 1) * R]).reshape(
            P, 2 * R
        )
        sx = np.ascontiguousarray(actx[:, ci * R : (ci + 1) * R])
        in_maps.append({"actb": sb, "actx": sx, "wpack": wpack})

    res = run_bass_kernel_spmd(nc, in_maps, core_ids=list(range(NCORES)), trace=TRACE)
    LAST_RESULTS = res

    out = np.empty((B, T, E), np.float32)
    for ci in range(NCORES):
        out[ci * BPC : (ci + 1) * BPC] = res.results[ci]["out"].reshape(BPC, T, E)
    return out



# revision 16
# speedup vs baseline: 1.4221x; 1.4221x over previous
"""Trainium2 Bass kernel for nn_ECA_69544110457542.

Math (per row r=(b,t)):
  dyn   = x[:, :31] @ Wd + bd
  value = x[:, 31] * Wv[0] + bv
  xhn   = [dyn | human@Wh+bh | nature@Wn+bn]                      (768 ch)
  pre_j = sum_k cw[t,k] * xhn[perm[ainv[j]+k-3]] + conv_b[t]      (j<256)
  sel   = softmax(relu(pre))
  out   = 0.5*(dyn*sel) @ Wvd1  +  0.5*dyn @ Wvd1 + value @ Wvd2 + bvd
          `------ device ------'  `------- folded into Wfold (host) -----'

Key design (v2):
  - the per-t conv-tap scales cw[t,k] are folded into FIVE host-prescaled
    copies of the activations (acts_k = acts * cw[t(row),k]); the PE then
    accumulates all 5 taps straight into ONE [128,256] PSUM region using
    the per-tap gathered weight matrices Wg_k.  This eliminates the wide
    1280-col PSUM drains AND the entire DVE conv-combine of v1;
  - conv_b is applied as the per-partition bias of an ACT Relu that also
    serves as the PSUM->SBUF drain; Exp (with free accum_out row-sum)
    follows on ACT from SBUF;
  - dyn and the folded linear output run as ONE fp32r matmul (full fp32
    precision at bf16 speed for >=256-col outputs);
  - z-path (transpose + z @ 0.5*Wvd1) is deferred one tile so PE never
    waits on the current tile's softmax chain; the PE stays continuously
    busy, which keeps its clock at the ramped 2.4 GHz p-state.

Sharding: pure data parallel, 32 batches per core on 8 cores.
"""

import sys

sys.path.insert(0, "/opt/trn_rl_repo")

from contextlib import ExitStack

import ml_dtypes
import numpy as np

import concourse.bass as bass
import concourse.tile as tile
from concourse import mybir
from concourse.tile import add_dep_helper
from concourse.bass_utils import run_bass_kernel_spmd

# problem constants
B, T, E = 256, 64, 256
XS, DS = 32, 31
HT, NT_ = 80, 80
C = 3 * E
KW = 5
NCORES = 8
BPC = B // NCORES          # 32 batches per core
R = BPC * T                # 2048 rows per core
P = 128
NTILES = R // P            # 16
AK = XS + 1 + HT + NT_     # 193 act rows: x(32) | ones | h(80) | n(80)
K2 = AK - 128              # 65
NG = KW * E                # 1280 gathered weight columns (5 taps x 256)

# packed-constants layout, fp32 slot offsets in [128, WPACK]
O_WG1 = 0                   # bf16 [128, 1280] -> 640 slots (taps, rows 0:128)
O_WG2 = O_WG1 + NG // 2     # 640:  bf16 [65, 1280] -> 640 slots (rows 128:193)
O_WV1 = O_WG2 + NG // 2     # 1280: bf16 0.5*Wvd1 [256, 256] -> 256 slots
O_IDB = O_WV1 + E           # 1536: bf16 identity [128,128] -> 64 slots
O_CB = O_IDB + P // 2       # 1600: fp32 [128, 1] conv_b per partition
WPACK = O_CB + 1            # 1601

F32 = mybir.dt.float32
F32R = mybir.dt.float32r
BF16 = mybir.dt.bfloat16
MULT = mybir.AluOpType.mult
ADD = mybir.AluOpType.add
RELU = mybir.ActivationFunctionType.Relu
EXP = mybir.ActivationFunctionType.Exp

_NC_CACHE = None
LAST_RESULTS = None
TRACE = False


def _patched_drain_and_barrier(self, tick_clock, wait_clock):
    # The stock kernel-tail drain puts every processor's final-tick wait on a
    # single Drain instruction; this walrus build rejects multi-wait
    # instructions, so spread the waits over a chain of drains instead.
    import bass_rust as _br
    from concourse.vector_clock import ScopedClock

    nc = self.nc
    drain_inst = nc.sync.drain()
    wait_clock.add_sem_waits(
        drain_inst.ins, ScopedClock({None: tick_clock.global_clock})
    )
    si = drain_inst.ins.sync_info
    if si is not None and len(si.on_wait) > 1:
        waits = list(si.on_wait)
        drain_inst.ins.sync_info = _br.SyncInfo(
            on_wait=[waits[0]], on_update=list(si.on_update)
        )
        for w in waits[1:]:
            d2 = nc.sync.drain()
            d2.ins.sync_info = _br.SyncInfo(on_wait=[w], on_update=[])
    nc.all_engine_barrier()
    assert self.sems is not None
    popped = nc._tile_sem_poison_stack.pop()
    assert popped is self._sem_poison
    nc.clear_and_free_semaphores(list(self.sems.allocated().values()))
    nc.all_engine_barrier()


tile.TileContext._drain_and_barrier = _patched_drain_and_barrier


def _build_nc():
    nc = bass.Bass()
    actA_d = nc.dram_tensor("actA", [P, KW * R], BF16, kind="ExternalInput")
    actB_d = nc.dram_tensor("actB", [K2, KW * R], BF16, kind="ExternalInput")
    actx_d = nc.dram_tensor("actx", [33, R], F32R, kind="ExternalInput")
    wdf_d = nc.dram_tensor("wdf", [33, 2 * E], F32R, kind="ExternalInput")
    wpack_d = nc.dram_tensor("wpack", [P, WPACK], F32, kind="ExternalInput")
    out_d = nc.dram_tensor("out", [R, E], F32, kind="ExternalOutput")
    actA3 = actA_d[:, :].rearrange("p (k r) -> p k r", k=KW)
    actB3 = actB_d[:, :].rearrange("p (k r) -> p k r", k=KW)

    with tile.TileContext(nc) as tc, ExitStack() as ctx:
        consts = ctx.enter_context(tc.tile_pool(name="consts", bufs=1))
        pactA = ctx.enter_context(tc.tile_pool(name="pactA", bufs=16))
        pactB = ctx.enter_context(tc.tile_pool(name="pactB", bufs=16))
        pactX = ctx.enter_context(tc.tile_pool(name="pactX", bufs=16))
        prel = ctx.enter_context(tc.tile_pool(name="prel", bufs=3))
        pex = ctx.enter_context(tc.tile_pool(name="pex", bufs=3))
        psml = ctx.enter_context(tc.tile_pool(name="psml", bufs=8))
        pz = ctx.enter_context(tc.tile_pool(name="pz", bufs=3))
        pzT = ctx.enter_context(tc.tile_pool(name="pzT", bufs=3))
        pot = ctx.enter_context(tc.tile_pool(name="pot", bufs=4))
        ptch = ctx.enter_context(tc.tile_pool(name="ptch", bufs=6))
        ppre = ctx.enter_context(tc.tile_pool(name="ppre", bufs=2, space="PSUM"))
        pdf = ctx.enter_context(tc.tile_pool(name="pdf", bufs=3, space="PSUM"))
        ptz = ctx.enter_context(tc.tile_pool(name="ptz", bufs=2, space="PSUM"))

        wp = consts.tile([P, WPACK], F32)
        nc.sync.dma_start(wp[:], wpack_d[:, :])
        wdfc = consts.tile([33, 2 * E], F32R)
        nc.sync.dma_start(wdfc[:], wdf_d[:, :])
        wpb = wp[:].bitcast(BF16)
        wg1 = wpb[:, 2 * O_WG1 : 2 * O_WG1 + NG]
        wg2 = wpb[0:K2, 2 * O_WG2 : 2 * O_WG2 + NG]
        wv1a = wpb[:, 2 * O_WV1 : 2 * O_WV1 + E]
        wv1b = wpb[:, 2 * O_WV1 + E : 2 * O_WV1 + 2 * E]
        identb = wpb[:, 2 * O_IDB : 2 * O_IDB + P]
        cb = wp[:, O_CB : O_CB + 1]

        # PE/DVE/ACT observe the weights DMA once (compute instructions carry
        # only ONE sem-wait on this walrus build)
        scr = ptz.tile([P, 1], F32, tag="ptz")
        nc.tensor.matmul(scr[:], identb, identb[:, 0:1], start=True, stop=True)
        nc.tensor.ldweights(wdfc[:].bitcast(BF16)[0:1, 0:2])
        wtouch = psml.tile([P, 1], F32, tag="sml")
        nc.vector.tensor_copy(wtouch[:], cb)
        stouch = psml.tile([P, 1], F32, tag="sml")
        nc.scalar.copy(stouch[:], cb)

        z_prev = None
        z_prev2 = None
        mmg_last_prev = None
        mmz_prev = None
        mmz_prev2 = None
        obufs = {}
        ob_hist = {}
        mmdf_hist = {}
        pend = None

        def flush_z(pend):
            # z-path of tile j, emitted one iteration later so PE never
            # waits on the current tile's softmax chain
            j, zj, pdfj = pend
            ptz_t = ptz.tile([P, 2, P], BF16, tag="ptz")
            nc.tensor.transpose(ptz_t[:, 0, :], zj[:, 0:128], identb)
            nc.tensor.transpose(ptz_t[:, 1, :], zj[:, 128:256], identb)
            zT = pzT.tile([P, 2, P], BF16)
            nc.vector.tensor_copy(zT[:], ptz_t[:])
            nc.tensor.matmul(
                pdfj[:, 256:512], zT[:, 0, :], wv1a,
                start=False, stop=False, skip_group_check=True,
            )
            mmz = nc.tensor.matmul(
                pdfj[:, 256:512], zT[:, 1, :], wv1b,
                start=False, stop=True, skip_group_check=True,
            )
            # ACT observes the gate's DVE tick before the out-copy
            zt_ = ptch.tile([1, 2], BF16, tag="tch2")
            nc.scalar.copy(zt_[:], zj[0:1, 0:2])
            if j % 4 == 0:
                obufs[j // 4] = pot.tile([P, 4, E], F32, tag="obuf",
                                         name=f"obuf{j // 4}")
            ob = obufs[j // 4]
            ob_hist[j] = nc.scalar.copy(ob[:, j % 4, :], pdfj[:, 256:512])
            if j % 4 == 3:
                g0 = (j - 3) * P
                odst = out_d[g0 : g0 + 4 * P, :].rearrange(
                    "(t p) e -> p t e", p=P
                )
                nc.gpsimd.dma_start(odst, ob[:])
            return mmz

        for i in range(NTILES):
            rows = slice(i * P, (i + 1) * P)
            aA = pactA.tile([P, KW, P], BF16)
            nc.sync.dma_start(aA[:], actA3[:, :, rows])
            aB = pactB.tile([K2, KW, P], BF16)
            nc.sync.dma_start(aB[:], actB3[:, :, rows])
            aX = pactX.tile([33, P], F32R)
            nc.sync.dma_start(aX[:], actx_d[:, rows])

            # "PE observes processor X" gadgets: every real matmul self-loads
            # its weights, so stray LDWEIGHTS are harmless
            absorbers = [
                nc.tensor.ldweights(aA[0:1, 0, 0:2]),
                nc.tensor.ldweights(aB[0:1, 0, 0:2]),
                nc.tensor.ldweights(aX[:].bitcast(BF16)[0:1, 0:2]),
            ]
            if z_prev2 is not None:
                absorbers.append(nc.tensor.ldweights(z_prev2[0:1, 0:2]))
            if (i - 3) in ob_hist:
                # absorb the pdf-PSUM recycle dep (read by ob of tile i-3)
                ldw_ob = nc.tensor.ldweights(wpb[0:1, 2:4])
                add_dep_helper(ldw_ob.ins, ob_hist[i - 3].ins, sync=True,
                               reason="absorb pdf recycle ACT wait")
                absorbers.append(ldw_ob)
            if mmg_last_prev is not None:
                ldw_self = nc.tensor.ldweights(wpb[0:1, 0:2])
                add_dep_helper(ldw_self.ins, mmg_last_prev.ins, sync=True,
                               reason="absorb PE W-W completion wait")
                if mmz_prev2 is not None:
                    add_dep_helper(ldw_self.ins, mmz_prev2.ins, sync=True,
                                   reason="absorb PE W-W completion wait")
                if (i - 3) in mmdf_hist:
                    add_dep_helper(ldw_self.ins, mmdf_hist[i - 3].ins,
                                   sync=True,
                                   reason="absorb pdf WAW completion wait")
                absorbers.append(ldw_self)

            # dyn (fp32r, cols 0:256) and folded-linear out part (cols 256:512)
            pdf_t = pdf.tile([P, 512], F32)
            mm_df = nc.tensor.matmul(
                pdf_t[:, 0:512], aX[:], wdfc[:],
                start=True, stop=True,
            )
            mmdf_hist[i] = mm_df
            for a in absorbers:
                add_dep_helper(mm_df.ins, a.ins, sync=False,
                               reason="absorbers run before first matmul")

            # gathered-conv accumulation: 5 taps x 2 K-chunks into one
            # [128,256] PSUM region (cw[t,k] is pre-folded into the acts)
            pre_t = ppre.tile([P, E], F32, tag="pre")
            for k in range(KW):
                mg = nc.tensor.matmul(
                    pre_t[:], aA[:, k, :], wg1[:, k * E : (k + 1) * E],
                    start=(k == 0), stop=False,
                )
                if k == 0:
                    for a in absorbers:
                        add_dep_helper(mg.ins, a.ins, sync=False,
                                       reason="absorbers first")
                mmg_last_prev = nc.tensor.matmul(
                    pre_t[:], aB[:, k, :], wg2[:, k * E : (k + 1) * E],
                    start=False, stop=(k == KW - 1),
                )

            # deferred z-path of the previous tile
            mmz_prev2 = mmz_prev
            if pend is not None:
                mmz_prev = flush_z(pend)

            # softmax chain for this tile (ACT: relu-drain + exp)
            rel = prel.tile([P, E], BF16, tag="rel")
            nc.scalar.activation(rel[:], pre_t[:], func=RELU, bias=cb)
            exm = pex.tile([P, E], BF16, tag="exm")
            ssum = psml.tile([P, 1], F32, tag="sml")
            nc.scalar.activation(exm[:], rel[:], func=EXP, accum_out=ssum[:])
            sinv = psml.tile([P, 1], F32, tag="sml")
            nc.vector.reciprocal(sinv[:], ssum[:])

            # DVE observes the dyn/fold matmul once before the gate
            pdtouch = ptch.tile([1, 2], BF16, tag="tch")
            nc.vector.tensor_copy(pdtouch[:], pdf_t[:].bitcast(BF16)[0:1, 0:2])

            # z = (exm / S) * dyn  (the 0.5 is folded into Wvd1)
            z = pz.tile([P, E], BF16, tag="z")
            nc.vector.scalar_tensor_tensor(
                z[:], exm[:], sinv[:], pdf_t[:, 0:E], op0=MULT, op1=MULT
            )
            z_prev2 = z_prev
            z_prev = z
            pend = (i, z, pdf_t)

        flush_z(pend)

    return nc


def _host_prep(x, human, nature, perm, Wv, bv, Wd, bd, Wh, bh, Wn, bn,
               conv_w, conv_b, Wvd, bvd):
    f = np.float32
    bf = ml_dtypes.bfloat16
    x = np.asarray(x, f)
    human = np.asarray(human, f)
    nature = np.asarray(nature, f)
    Wv = np.asarray(Wv, f); bv = np.asarray(bv, f)
    Wd = np.asarray(Wd, f); bd = np.asarray(bd, f)
    Wh = np.asarray(Wh, f); bh = np.asarray(bh, f)
    Wn = np.asarray(Wn, f); bn = np.asarray(bn, f)
    conv_w = np.asarray(conv_w, f)
    conv_b = np.asarray(conv_b, f)
    Wvd = np.asarray(Wvd, f); bvd = np.asarray(bvd, f)
    perm = np.asarray(perm).astype(np.int64)

    Wvd1 = Wvd[:E, :]
    Wvd2 = Wvd[E:, :]
    BT = B * T

    acts = np.concatenate(
        [
            x.reshape(BT, XS),
            np.ones((BT, 1), f),
            human.reshape(BT, HT),
            nature.reshape(BT, NT_),
        ],
        axis=1,
    )
    actsT = np.ascontiguousarray(acts.T)  # [193, BT] fp32

    # five tap-scaled copies: acts_k[:, r] = acts[:, r] * conv_w[t(r), 0, k]
    sc = np.empty((KW, BT), f)
    for k in range(KW):
        sc[k] = np.tile(conv_w[:, 0, k], B)
    scaled = actsT[None, :, :] * sc[:, None, :]          # [5, 193, BT]
    actA = np.ascontiguousarray(
        scaled[:, 0:128, :].transpose(1, 0, 2)).astype(bf)   # [128, 5, BT]
    actB = np.ascontiguousarray(
        scaled[:, 128:AK, :].transpose(1, 0, 2)).astype(bf)  # [65, 5, BT]
    actx = np.ascontiguousarray(actsT[0:33])             # fp32 [33, BT]

    wpack = np.zeros((P, WPACK), f)
    wpv = wpack.view(bf)  # bf16 alias [128, 2*WPACK]

    # dyn + folded linear path (fp32, one fp32r matmul)
    wdf = np.zeros((33, 2 * E), f)
    wdf[0:DS, 0:E] = Wd
    wdf[32, 0:E] = bd
    wdf[0:DS, E:] = 0.5 * (Wd @ Wvd1)
    wdf[31, E:] = Wv[0] @ Wvd2
    wdf[32, E:] = 0.5 * (bd @ Wvd1) + bv @ Wvd2 + bvd

    # per-tap gathered conv weights (bf16): Wg[:, k*256+j] reproduces
    # xhn[:, perm[ainv[j]+k-3]] (incl. its bias on the ones-row 32)
    ainv = np.argsort(perm)
    Wg = np.zeros((AK, NG), f)
    for k in range(KW):
        pos = ainv[:E] + k - 3
        for j in range(E):
            pj = pos[j]
            if 0 <= pj < C:
                c = perm[pj]
                col = k * E + j
                if c < E:
                    Wg[0:DS, col] = Wd[:, c]
                    Wg[32, col] = bd[c]
                elif c < 2 * E:
                    Wg[33:113, col] = Wh[:, c - E]
                    Wg[32, col] = bh[c - E]
                else:
                    Wg[113:193, col] = Wn[:, c - 2 * E]
                    Wg[32, col] = bn[c - 2 * E]
    wpv[:, 2 * O_WG1 : 2 * O_WG1 + NG] = Wg[0:128].astype(bf)
    wpv[0:K2, 2 * O_WG2 : 2 * O_WG2 + NG] = Wg[128:AK].astype(bf)

    # 0.5 * Wvd1 (bf16), split into two K-chunks
    wv1 = (0.5 * Wvd1).astype(bf)
    wpv[:, 2 * O_WV1 : 2 * O_WV1 + E] = wv1[0:128]
    wpv[:, 2 * O_WV1 + E : 2 * O_WV1 + 2 * E] = wv1[128:256]

    wpv[:, 2 * O_IDB : 2 * O_IDB + P] = np.eye(P, dtype=bf)
    wpack[:, O_CB] = np.tile(conv_b, 2)
    return actA, actB, actx, wdf, wpack


def kernel(**inputs):
    global _NC_CACHE, LAST_RESULTS
    actA, actB, actx, wdf, wpack = _host_prep(**inputs)

    if _NC_CACHE is None:
        _NC_CACHE = _build_nc()
    nc = _NC_CACHE

    in_maps = []
    for ci in range(NCORES):
        rs = slice(ci * R, (ci + 1) * R)
        sA = np.ascontiguousarray(actA[:, :, rs]).reshape(P, KW * R)
        sB = np.ascontiguousarray(actB[:, :, rs]).reshape(K2, KW * R)
        sx = np.ascontiguousarray(actx[:, rs])
        in_maps.append({"actA": sA, "actB": sB, "actx": sx, "wdf": wdf,
                        "wpack": wpack})

    res = run_bass_kernel_spmd(nc, in_maps, core_ids=list(range(NCORES)),
                               trace=TRACE)
    LAST_RESULTS = res

    out = np.empty((B, T, E), np.float32)
    for ci in range(NCORES):
        out[ci * BPC : (ci + 1) * BPC] = res.results[ci]["out"].reshape(
            BPC, T, E)
    return out


# revision 26
# speedup vs baseline: 1.5179x; 1.0674x over previous
"""Trainium2 Bass kernel for nn_ECA_69544110457542.

Math (per row r=(b,t)):
  dyn   = x[:, :31] @ Wd + bd
  value = x[:, 31] * Wv[0] + bv
  xhn   = [dyn | human@Wh+bh | nature@Wn+bn]                      (768 ch)
  pre_j = sum_k cw[t,k] * xhn[perm[ainv[j]+k-3]] + conv_b[t]      (j<256)
  sel   = softmax(relu(pre))
  out   = 0.5*(dyn*sel) @ Wvd1  +  0.5*dyn @ Wvd1 + value @ Wvd2 + bvd
          `------ device ------'  `------- folded into Wfold (host) -----'

Key design (v3):
  - per-t conv-tap scales cw[t,k] are folded into FIVE host-prescaled fp8
    copies of the activations; the PE accumulates all 5 taps straight into
    ONE [128,256] PSUM region using per-tap gathered fp8 weights Wg_k.
    (softmax path is insensitive: fp8 G costs < 1e-4 of output scale);
  - conv_b rides the per-partition bias of an ACT Relu that is also the
    PSUM->SBUF drain; Exp (free accum_out row-sum) follows on ACT;
  - dyn + the folded linear output are ONE fp32r matmul (full fp32
    precision at ~bf16 speed for wide outputs);
  - z-path is split: transpose at depth 1, z @ 0.5*Wvd1 + out-drain at
    depth 2, so PE never waits on a softmax chain and its clock ramps to
    the 2.4 GHz p-state;
  - DMAs: per-tile-contiguous layouts (fat descriptors), priority-ordered
    const loads, issue spread over SP (acts/consts), DVE (actB) and
    GpSimd (per-tile output) sequencers; all SBUF pools sized to never
    recycle so no extra sem-waits appear.

Sharding: pure data parallel, 32 batches per core on 8 cores.
"""

import sys

sys.path.insert(0, "/opt/trn_rl_repo")

from contextlib import ExitStack

import ml_dtypes
import numpy as np

import concourse.bass as bass
import concourse.tile as tile
from concourse import mybir
from concourse.tile import add_dep_helper
from concourse.bass_utils import run_bass_kernel_spmd

# problem constants
B, T, E = 256, 64, 256
XS, DS = 32, 31
HT, NT_ = 80, 80
C = 3 * E
KW = 5
NCORES = 8
BPC = B // NCORES          # 32 batches per core
R = BPC * T                # 2048 rows per core
P = 128
NTILES = R // P            # 16
AK = XS + 1 + HT + NT_     # 193 act rows: x(32) | ones | h(80) | n(80)
K2 = AK - 128              # 65
NG = KW * E                # 1280 gathered weight columns (5 taps x 256)

# packed small consts, fp32 slot offsets in [128, WPK2]
O_WV1 = 0                   # bf16 0.5*Wvd1 [256, 256] -> 256 slots
O_IDB = O_WV1 + E           # 256: bf16 identity [128,128] -> 64 slots
O_CB = O_IDB + P // 2       # 320: fp32 [128, 1] conv_b per partition
WPK2 = O_CB + 1             # 321

F32 = mybir.dt.float32
F32R = mybir.dt.float32r
BF16 = mybir.dt.bfloat16
FP8 = mybir.dt.float8e4
MULT = mybir.AluOpType.mult
RELU = mybir.ActivationFunctionType.Relu
EXP = mybir.ActivationFunctionType.Exp

_NC_CACHE = None
LAST_RESULTS = None
TRACE = False


def _patched_drain_and_barrier(self, tick_clock, wait_clock):
    # The stock kernel-tail drain puts every processor's final-tick wait on a
    # single Drain instruction; this walrus build rejects multi-wait
    # instructions, so spread the waits over a chain of drains instead.
    import bass_rust as _br
    from concourse.vector_clock import ScopedClock

    nc = self.nc
    drain_inst = nc.sync.drain()
    wait_clock.add_sem_waits(
        drain_inst.ins, ScopedClock({None: tick_clock.global_clock})
    )
    si = drain_inst.ins.sync_info
    if si is not None and len(si.on_wait) > 1:
        waits = list(si.on_wait)
        drain_inst.ins.sync_info = _br.SyncInfo(
            on_wait=[waits[0]], on_update=list(si.on_update)
        )
        for w in waits[1:]:
            d2 = nc.sync.drain()
            d2.ins.sync_info = _br.SyncInfo(on_wait=[w], on_update=[])
    nc.all_engine_barrier()
    assert self.sems is not None
    popped = nc._tile_sem_poison_stack.pop()
    assert popped is self._sem_poison
    nc.clear_and_free_semaphores(list(self.sems.allocated().values()))
    nc.all_engine_barrier()


tile.TileContext._drain_and_barrier = _patched_drain_and_barrier


def _build_nc():
    nc = bass.Bass()
    actA_d = nc.dram_tensor("actA", [P, NTILES * KW * P], FP8,
                            kind="ExternalInput")
    actB_d = nc.dram_tensor("actB", [K2, NTILES * KW * P], FP8,
                            kind="ExternalInput")
    actx_d = nc.dram_tensor("actx", [33, R], F32R, kind="ExternalInput")
    wdf_d = nc.dram_tensor("wdf", [33, 2 * E], F32R, kind="ExternalInput")
    wg1_d = nc.dram_tensor("wg1", [P, NG], FP8, kind="ExternalInput")
    wg2_d = nc.dram_tensor("wg2", [K2, NG], FP8, kind="ExternalInput")
    wpk2_d = nc.dram_tensor("wpk2", [P, WPK2], F32, kind="ExternalInput")
    out_d = nc.dram_tensor("out", [R, E], F32, kind="ExternalOutput")
    actA4 = actA_d[:, :].rearrange("p (i k r) -> p i k r", i=NTILES, k=KW)
    actB4 = actB_d[:, :].rearrange("p (i k r) -> p i k r", i=NTILES, k=KW)

    with tile.TileContext(nc) as tc, ExitStack() as ctx:
        consts = ctx.enter_context(tc.tile_pool(name="consts", bufs=1))
        pactA = ctx.enter_context(tc.tile_pool(name="pactA", bufs=16))
        pactB = ctx.enter_context(tc.tile_pool(name="pactB", bufs=16))
        pactX = ctx.enter_context(tc.tile_pool(name="pactX", bufs=4))
        prel = ctx.enter_context(tc.tile_pool(name="prel", bufs=3))
        pex = ctx.enter_context(tc.tile_pool(name="pex", bufs=3))
        psml = ctx.enter_context(tc.tile_pool(name="psml", bufs=8))
        pz = ctx.enter_context(tc.tile_pool(name="pz", bufs=3))
        pzT = ctx.enter_context(tc.tile_pool(name="pzT", bufs=3))
        pot = ctx.enter_context(tc.tile_pool(name="pot", bufs=8))
        ptch = ctx.enter_context(tc.tile_pool(name="ptch", bufs=6))
        ppre = ctx.enter_context(tc.tile_pool(name="ppre", bufs=2, space="PSUM"))
        pdf = ctx.enter_context(tc.tile_pool(name="pdf", bufs=4, space="PSUM"))
        ptz = ctx.enter_context(tc.tile_pool(name="ptz", bufs=2, space="PSUM"))

        # priority-ordered const loads: wdf (tile-0 mm_df) first, then the
        # G weights, then the relaxed pack (wv1/identity/cb, first use ~iter1)
        wdfc = consts.tile([33, 2 * E], F32R)
        nc.sync.dma_start(wdfc[:], wdf_d[:, :])
        wg1c = consts.tile([P, NG], FP8)
        nc.sync.dma_start(wg1c[0:64, :], wg1_d[0:64, :])
        dwg1b = nc.sync.dma_start(wg1c[64:128, :], wg1_d[64:128, :])
        wg2c = consts.tile([K2, NG], FP8)
        nc.sync.dma_start(wg2c[:], wg2_d[:, :])
        wp2 = consts.tile([P, WPK2], F32)
        nc.sync.dma_start(wp2[:], wpk2_d[:, :])
        wpb = wp2[:].bitcast(BF16)
        wv1a = wpb[:, 2 * O_WV1 : 2 * O_WV1 + E]
        wv1b = wpb[:, 2 * O_WV1 + E : 2 * O_WV1 + 2 * E]
        identb = wpb[:, 2 * O_IDB : 2 * O_IDB + P]
        cb = wp2[:, O_CB : O_CB + 1]

        # each engine observes every const DMA once up front (compute
        # instructions carry only ONE sem-wait on this walrus build)
        nc.tensor.ldweights(wdfc[:].bitcast(BF16)[0:1, 0:2])
        nc.tensor.ldweights(wg1c[0:1, 0:2])
        ldw1b = nc.tensor.ldweights(wg1c[0:1, 2:4])
        add_dep_helper(ldw1b.ins, dwg1b.ins, sync=True,
                       reason="PE observes upper-half wg1 DMA")
        nc.tensor.ldweights(wg2c[0:1, 0:2])
        scr = ptz.tile([P, 1], F32, tag="ptz")
        nc.tensor.matmul(scr[:], identb, identb[:, 0:1], start=True, stop=True)
        wtouch = psml.tile([P, 1], F32, tag="sml")
        nc.vector.tensor_copy(wtouch[:], cb)
        stouch = psml.tile([P, 1], F32, tag="sml")
        nc.scalar.copy(stouch[:], cb)

        z_hist = {}
        pdf_hist = {}
        zT_hist = {}
        ob_hist = {}
        obufs = {}
        mmdf_hist = {}
        mmg_last_prev = None
        mmz_prev = None
        mmz_prev2 = None

        def flush_T(j):
            # transpose of tile j's gate output, one iteration after tile j
            zj = z_hist[j]
            ptz_t = ptz.tile([P, 2, P], BF16, tag="ptz")
            nc.tensor.transpose(ptz_t[:, 0, :], zj[:, 0:128], identb)
            nc.tensor.transpose(ptz_t[:, 1, :], zj[:, 128:256], identb)
            zT = pzT.tile([P, 2, P], BF16)
            nc.vector.tensor_copy(zT[:], ptz_t[:])
            zT_hist[j] = zT

        def flush_zmm(j):
            # z @ 0.5*Wvd1 accumulated onto the folded output, then drained
            # and written out; two iterations after tile j
            pdfj = pdf_hist[j]
            zT = zT_hist[j]
            nc.tensor.matmul(
                pdfj[:, 256:512], zT[:, 0, :], wv1a,
                start=False, stop=False, skip_group_check=True,
            )
            mmz = nc.tensor.matmul(
                pdfj[:, 256:512], zT[:, 1, :], wv1b,
                start=False, stop=True, skip_group_check=True,
            )
            if j % 2 == 0:
                obufs[j // 2] = pot.tile([P, 2, E], F32, tag="obuf",
                                         name=f"obuf{j // 2}")
            ob = obufs[j // 2]
            # ACT observes the zT-copy tick so the ob drain carries a single
            # (PE) wait
            ztch = ptch.tile([1, 2], BF16, tag="tch2")
            nc.scalar.copy(ztch[:], zT[0:1, 0, 0:2])
            ob_hist[j] = nc.scalar.copy(ob[:, j % 2, :], pdfj[:, 256:512])
            if j % 2 == 1:
                g0 = (j - 1) * P
                odst = out_d[g0 : g0 + 2 * P, :].rearrange(
                    "(t p) e -> p t e", p=P)
                nc.gpsimd.dma_start(odst, ob[:])
            return mmz

        for i in range(NTILES):
            aA = pactA.tile([P, KW, P], FP8)
            nc.sync.dma_start(aA[:], actA4[:, i, :, :])
            aB = pactB.tile([K2, KW, P], FP8)
            nc.scalar.dma_start(aB[:], actB4[:, i, :, :])
            if i % 4 == 0:
                aX4 = pactX.tile([33, 4, P], F32R)
                nc.sync.dma_start(
                    aX4[:], actx_d[:, i * P : (i + 4) * P].rearrange(
                        "p (f r) -> p f r", f=4)
                )
            aX = aX4[:, i % 4, :]

            # "PE observes processor X" gadgets: every real matmul self-loads
            # its weights, so stray LDWEIGHTS are harmless
            absorbers = [
                nc.tensor.ldweights(aA[0:1, 0, 0:2]),
                nc.tensor.ldweights(aB[0:1, 0, 0:2]),
            ]
            if i % 4 == 0:
                absorbers.append(
                    nc.tensor.ldweights(aX4[:].bitcast(BF16)[0:1, 0, 0:2]))
            if (i - 4) in ob_hist:
                # absorb the pdf-PSUM recycle dep (read by ob of tile i-4)
                ldw_ob = nc.tensor.ldweights(wpb[0:1, 2:4])
                add_dep_helper(ldw_ob.ins, ob_hist[i - 4].ins, sync=True,
                               reason="absorb pdf recycle ACT wait")
                absorbers.append(ldw_ob)
            if mmg_last_prev is not None:
                ldw_self = nc.tensor.ldweights(wpb[0:1, 0:2])
                add_dep_helper(ldw_self.ins, mmg_last_prev.ins, sync=True,
                               reason="absorb PE W-W completion wait")
                if mmz_prev2 is not None:
                    add_dep_helper(ldw_self.ins, mmz_prev2.ins, sync=True,
                                   reason="absorb PE W-W completion wait")
                if (i - 4) in mmdf_hist:
                    add_dep_helper(ldw_self.ins, mmdf_hist[i - 4].ins,
                                   sync=True,
                                   reason="absorb pdf WAW completion wait")
                absorbers.append(ldw_self)

            # dyn (cols 0:256) and folded-linear out part (cols 256:512),
            # one fp32r matmul
            pdf_t = pdf.tile([P, 512], F32)
            mm_df = nc.tensor.matmul(
                pdf_t[:, 0:512], aX, wdfc[:], start=True, stop=True,
            )
            mmdf_hist[i] = mm_df
            pdf_hist[i] = pdf_t
            for a in absorbers:
                add_dep_helper(mm_df.ins, a.ins, sync=False,
                               reason="absorbers run before first matmul")

            # gathered-conv accumulation: 5 taps x 2 K-chunks into one
            # [128,256] PSUM region (cw[t,k] is pre-folded into the acts)
            pre_t = ppre.tile([P, E], F32, tag="pre")
            for k in range(KW):
                mg = nc.tensor.matmul(
                    pre_t[:], aA[:, k, :], wg1c[:, k * E : (k + 1) * E],
                    start=(k == 0), stop=False,
                )
                if k == 0:
                    for a in absorbers:
                        add_dep_helper(mg.ins, a.ins, sync=False,
                                       reason="absorbers first")
                mmg_last_prev = nc.tensor.matmul(
                    pre_t[:], aB[:, k, :], wg2c[:, k * E : (k + 1) * E],
                    start=False, stop=(k == KW - 1),
                )

            # deferred z-path: transpose of tile i-1, zmm+out of tile i-2
            if i >= 1:
                flush_T(i - 1)
            mmz_prev2 = mmz_prev
            if i >= 2:
                mmz_prev = flush_zmm(i - 2)

            # softmax chain for this tile (ACT: relu-drain + exp)
            rel = prel.tile([P, E], BF16, tag="rel")
            nc.scalar.activation(rel[:], pre_t[:], func=RELU, bias=cb)
            exm = pex.tile([P, E], BF16, tag="exm")
            ssum = psml.tile([P, 1], F32, tag="sml")
            nc.scalar.activation(exm[:], rel[:], func=EXP, accum_out=ssum[:])
            sinv = psml.tile([P, 1], F32, tag="sml")
            nc.vector.reciprocal(sinv[:], ssum[:])

            # DVE observes the dyn/fold matmul once before the gate
            pdtouch = ptch.tile([1, 2], BF16, tag="tch")
            nc.vector.tensor_copy(pdtouch[:], pdf_t[:].bitcast(BF16)[0:1, 0:2])

            # z = (exm / S) * dyn  (the 0.5 is folded into Wvd1)
            z = pz.tile([P, E], BF16, tag="z")
            nc.vector.scalar_tensor_tensor(
                z[:], exm[:], sinv[:], pdf_t[:, 0:E], op0=MULT, op1=MULT
            )
            z_hist[i] = z

        flush_T(NTILES - 1)
        mmz_prev2 = mmz_prev
        mmz_prev = flush_zmm(NTILES - 2)
        flush_zmm(NTILES - 1)

    return nc


def _host_prep(x, human, nature, perm, Wv, bv, Wd, bd, Wh, bh, Wn, bn,
               conv_w, conv_b, Wvd, bvd):
    f = np.float32
    bf = ml_dtypes.bfloat16
    f8 = ml_dtypes.float8_e4m3fn
    x = np.asarray(x, f)
    human = np.asarray(human, f)
    nature = np.asarray(nature, f)
    Wv = np.asarray(Wv, f); bv = np.asarray(bv, f)
    Wd = np.asarray(Wd, f); bd = np.asarray(bd, f)
    Wh = np.asarray(Wh, f); bh = np.asarray(bh, f)
    Wn = np.asarray(Wn, f); bn = np.asarray(bn, f)
    conv_w = np.asarray(conv_w, f)
    conv_b = np.asarray(conv_b, f)
    Wvd = np.asarray(Wvd, f); bvd = np.asarray(bvd, f)
    perm = np.asarray(perm).astype(np.int64)

    Wvd1 = Wvd[:E, :]
    Wvd2 = Wvd[E:, :]
    BT = B * T

    acts = np.concatenate(
        [
            x.reshape(BT, XS),
            np.ones((BT, 1), f),
            human.reshape(BT, HT),
            nature.reshape(BT, NT_),
        ],
        axis=1,
    )
    actsT = np.ascontiguousarray(acts.T)  # [193, BT] fp32

    # five tap-scaled fp8 copies: acts_k[:, r] = acts[:, r] * conv_w[t(r),0,k]
    sc = np.empty((KW, BT), f)
    for k in range(KW):
        sc[k] = np.tile(conv_w[:, 0, k], B)
    scaled = actsT[None, :, :] * sc[:, None, :]          # [5, 193, BT]
    actA = np.ascontiguousarray(
        scaled[:, 0:128, :].transpose(1, 0, 2)).astype(f8)   # [128, 5, BT]
    actB = np.ascontiguousarray(
        scaled[:, 128:AK, :].transpose(1, 0, 2)).astype(f8)  # [65, 5, BT]
    actx = np.ascontiguousarray(actsT[0:33])             # fp32 [33, BT]

    # dyn + folded linear path (fp32, one fp32r matmul)
    wdf = np.zeros((33, 2 * E), f)
    wdf[0:DS, 0:E] = Wd
    wdf[32, 0:E] = bd
    wdf[0:DS, E:] = 0.5 * (Wd @ Wvd1)
    wdf[31, E:] = Wv[0] @ Wvd2
    wdf[32, E:] = 0.5 * (bd @ Wvd1) + bv @ Wvd2 + bvd

    # per-tap gathered conv weights (fp8): Wg[:, k*256+j] reproduces
    # xhn[:, perm[ainv[j]+k-3]] (incl. its bias on the ones-row 32)
    ainv = np.argsort(perm)
    Wg = np.zeros((AK, NG), f)
    for k in range(KW):
        pos = ainv[:E] + k - 3
        for j in range(E):
            pj = pos[j]
            if 0 <= pj < C:
                c = perm[pj]
                col = k * E + j
                if c < E:
                    Wg[0:DS, col] = Wd[:, c]
                    Wg[32, col] = bd[c]
                elif c < 2 * E:
                    Wg[33:113, col] = Wh[:, c - E]
                    Wg[32, col] = bh[c - E]
                else:
                    Wg[113:193, col] = Wn[:, c - 2 * E]
                    Wg[32, col] = bn[c - 2 * E]
    wg1 = Wg[0:128].astype(f8)
    wg2 = Wg[128:AK].astype(f8)

    wpk2 = np.zeros((P, WPK2), f)
    wpv = wpk2.view(bf)
    wv1 = (0.5 * Wvd1).astype(bf)
    wpv[:, 2 * O_WV1 : 2 * O_WV1 + E] = wv1[0:128]
    wpv[:, 2 * O_WV1 + E : 2 * O_WV1 + 2 * E] = wv1[128:256]
    wpv[:, 2 * O_IDB : 2 * O_IDB + P] = np.eye(P, dtype=bf)
    wpk2[:, O_CB] = np.tile(conv_b, 2)
    return actA, actB, actx, wdf, wg1, wg2, wpk2


def kernel(**inputs):
    global _NC_CACHE, LAST_RESULTS
    actA, actB, actx, wdf, wg1, wg2, wpk2 = _host_prep(**inputs)

    if _NC_CACHE is None:
        _NC_CACHE = _build_nc()
    nc = _NC_CACHE

    in_maps = []
    for ci in range(NCORES):
        rs = slice(ci * R, (ci + 1) * R)
        # [p, 5, R_core] -> per-tile-contiguous [p, i, k, 128]
        sA = np.ascontiguousarray(
            actA[:, :, rs].reshape(P, KW, NTILES, P).transpose(0, 2, 1, 3)
        ).reshape(P, NTILES * KW * P)
        sB = np.ascontiguousarray(
            actB[:, :, rs].reshape(K2, KW, NTILES, P).transpose(0, 2, 1, 3)
        ).reshape(K2, NTILES * KW * P)
        sx = np.ascontiguousarray(actx[:, rs])
        in_maps.append({"actA": sA, "actB": sB, "actx": sx, "wdf": wdf,
                        "wg1": wg1, "wg2": wg2, "wpk2": wpk2})

    res = run_bass_kernel_spmd(nc, in_maps, core_ids=list(range(NCORES)),
                               trace=TRACE)
    LAST_RESULTS = res

    out = np.empty((B, T, E), np.float32)
    for ci in range(NCORES):
        out[ci * BPC : (ci + 1) * BPC] = res.results[ci]["out"].reshape(
            BPC, T, E)
    return out


# revision 32
# speedup vs baseline: 2.0371x; 1.3420x over previous
"""Trainium2 Bass kernel for nn_ECA_69544110457542.

Math (per row r=(b,t)):
  dyn   = x[:, :31] @ Wd + bd
  value = x[:, 31] * Wv[0] + bv
  xhn   = [dyn | human@Wh+bh | nature@Wn+bn]                      (768 ch)
  pre_j = sum_k cw[t,k] * xhn[perm[ainv[j]+k-3]] + conv_b[t]      (j<256)
  sel   = softmax(relu(pre))
  out   = 0.5*(dyn*sel) @ Wvd1  +  0.5*dyn @ Wvd1 + value @ Wvd2 + bvd
          `------ device ------'  `------- folded into Wfold (host) -----'

Key design (v4):
  - per-t conv-tap scales cw[t,k] are folded into host-prescaled fp8
    copies of the activations, and all 5 taps are STACKED into one
    965-row (pad 1024) contraction; the PE computes `pre` with FOUR
    fp8 DoubleRow matmuls (2 K-groups each) into one [128,256] PSUM
    region.  The softmax path is insensitive to fp8 noise;
  - conv_b rides the per-partition bias of an ACT Relu that is also the
    PSUM->SBUF drain; Exp (free accum_out row-sum) follows on ACT;
  - dyn + the folded linear output are ONE bf16 [33x512] matmul;
  - z-path: transpose (bf16) at depth 1 with the PSUM->SBUF copy casting
    to fp8, then a single fp8 DoubleRow matmul z @ 0.5*Wvd1 at depth 2.
    PE never waits on a softmax chain;
  - DMAs: per-tile-contiguous fp8 activations, priority-ordered const
    loads, issue spread over SP / ACT / GpSimd sequencers, pools sized
    to never recycle within the DMA-visible window.

Sharding: pure data parallel, 32 batches per core on 8 cores.
"""

import sys

sys.path.insert(0, "/opt/trn_rl_repo")

from contextlib import ExitStack

import ml_dtypes
import numpy as np

import concourse.bass as bass
import concourse.tile as tile
from concourse import mybir
from concourse.tile import add_dep_helper
from concourse.bass_utils import run_bass_kernel_spmd

# problem constants
B, T, E = 256, 64, 256
XS, DS = 32, 31
HT, NT_ = 80, 80
C = 3 * E
KW = 5
NCORES = 8
BPC = B // NCORES          # 32 batches per core
R = BPC * T                # 2048 rows per core
P = 128
NTILES = R // P            # 16
AK = XS + 1 + HT + NT_     # 193 act rows: x(32) | ones | h(80) | n(80)
SK = KW * AK               # 965 stacked contraction rows
SKP = 1024                 # padded to 4 DoubleRow chunks of 256
NC_ = SKP // 256           # 4 chunks

F32 = mybir.dt.float32
BF16 = mybir.dt.bfloat16
FP8 = mybir.dt.float8e4
MULT = mybir.AluOpType.mult
RELU = mybir.ActivationFunctionType.Relu
EXP = mybir.ActivationFunctionType.Exp
DR = mybir.MatmulPerfMode.DoubleRow

_NC_CACHE = None
LAST_RESULTS = None
TRACE = False


def _patched_drain_and_barrier(self, tick_clock, wait_clock):
    # The stock kernel-tail drain puts every processor's final-tick wait on a
    # single Drain instruction; this walrus build rejects multi-wait
    # instructions, so spread the waits over a chain of drains instead.
    import bass_rust as _br
    from concourse.vector_clock import ScopedClock

    nc = self.nc
    drain_inst = nc.sync.drain()
    wait_clock.add_sem_waits(
        drain_inst.ins, ScopedClock({None: tick_clock.global_clock})
    )
    si = drain_inst.ins.sync_info
    if si is not None and len(si.on_wait) > 1:
        waits = list(si.on_wait)
        drain_inst.ins.sync_info = _br.SyncInfo(
            on_wait=[waits[0]], on_update=list(si.on_update)
        )
        for w in waits[1:]:
            d2 = nc.sync.drain()
            d2.ins.sync_info = _br.SyncInfo(on_wait=[w], on_update=[])
    nc.all_engine_barrier()
    assert self.sems is not None
    popped = nc._tile_sem_poison_stack.pop()
    assert popped is self._sem_poison
    nc.clear_and_free_semaphores(list(self.sems.allocated().values()))
    nc.all_engine_barrier()


tile.TileContext._drain_and_barrier = _patched_drain_and_barrier


def _build_nc():
    nc = bass.Bass()
    actS_d = nc.dram_tensor("actS", [P, NTILES * NC_ * 2 * P], FP8,
                            kind="ExternalInput")
    actd_d = nc.dram_tensor("actd", [33, R], BF16, kind="ExternalInput")
    wdf_d = nc.dram_tensor("wdf", [33, 2 * E], BF16, kind="ExternalInput")
    wgs_d = nc.dram_tensor("wgs", [P, NC_ * 2 * E], FP8, kind="ExternalInput")
    wv18_d = nc.dram_tensor("wv18", [P, 2 * E], FP8, kind="ExternalInput")
    idb_d = nc.dram_tensor("idb", [P, P], BF16, kind="ExternalInput")
    cb_d = nc.dram_tensor("cb", [P, 1], F32, kind="ExternalInput")
    out_d = nc.dram_tensor("out", [R, E], F32, kind="ExternalOutput")
    actS5 = actS_d[:, :].rearrange("p (i c o r) -> p i c o r",
                                   i=NTILES, c=NC_, o=2)

    with tile.TileContext(nc) as tc, ExitStack() as ctx:
        consts = ctx.enter_context(tc.tile_pool(name="consts", bufs=1))
        pactS = ctx.enter_context(tc.tile_pool(name="pactS", bufs=16))
        pactX = ctx.enter_context(tc.tile_pool(name="pactX", bufs=4))
        prel = ctx.enter_context(tc.tile_pool(name="prel", bufs=16))
        pex = ctx.enter_context(tc.tile_pool(name="pex", bufs=16))
        psml = ctx.enter_context(tc.tile_pool(name="psml", bufs=40))
        pz = ctx.enter_context(tc.tile_pool(name="pz", bufs=16))
        pzT = ctx.enter_context(tc.tile_pool(name="pzT", bufs=16))
        pot = ctx.enter_context(tc.tile_pool(name="pot", bufs=8))
        ptch = ctx.enter_context(tc.tile_pool(name="ptch", bufs=40))
        ppre = ctx.enter_context(tc.tile_pool(name="ppre", bufs=2, space="PSUM"))
        pdf = ctx.enter_context(tc.tile_pool(name="pdf", bufs=4, space="PSUM"))
        ptz = ctx.enter_context(tc.tile_pool(name="ptz", bufs=2, space="PSUM"))

        # priority-ordered const loads: wdf (tile-0 mm_df) first, then the
        # stacked G weights, then the relaxed ones (wv18/identity/cb)
        wdfc = consts.tile([33, 2 * E], BF16)
        nc.sync.dma_start(wdfc[:], wdf_d[:, :])
        wgsc = consts.tile([P, NC_, 2, E], FP8)
        wgs4 = wgs_d[:, :].rearrange("p (c o e) -> p c o e", c=NC_, o=2)
        nc.sync.dma_start(wgsc[0:64, :, :, :], wgs4[0:64, :, :, :])
        dwgb = nc.sync.dma_start(wgsc[64:128, :, :, :], wgs4[64:128, :, :, :])
        wv18c = consts.tile([P, 2, E], FP8)
        nc.sync.dma_start(wv18c[:], wv18_d[:, :].rearrange(
            "p (o e) -> p o e", o=2))
        idbc = consts.tile([P, P], BF16)
        nc.sync.dma_start(idbc[:], idb_d[:, :])
        cbc = consts.tile([P, 1], F32)
        nc.sync.dma_start(cbc[:], cb_d[:, :])

        # each engine observes every const DMA once up front (compute
        # instructions carry only ONE sem-wait on this walrus build)
        nc.tensor.ldweights(wdfc[0:1, 0:2])
        nc.tensor.ldweights(wgsc[0:1, 0, 0, 0:2])
        ldwb = nc.tensor.ldweights(wgsc[0:1, 0, 0, 2:4])
        add_dep_helper(ldwb.ins, dwgb.ins, sync=True,
                       reason="PE observes upper-half wgs DMA")
        nc.tensor.ldweights(wv18c[0:1, 0, 0:2])
        scr = ptz.tile([P, 1], F32, tag="ptz")
        nc.tensor.matmul(scr[:], idbc[:], idbc[:, 0:1], start=True, stop=True)
        wtouch = psml.tile([P, 1], F32, tag="sml")
        nc.vector.tensor_copy(wtouch[:], cbc[:])
        stouch = psml.tile([P, 1], F32, tag="sml")
        nc.scalar.copy(stouch[:], cbc[:])

        z_hist = {}
        pdf_hist = {}
        zT_hist = {}
        ob_hist = {}
        obufs = {}
        mmdf_hist = {}
        mmg_last_prev = None
        mmz_prev = None
        mmz_prev2 = None

        def flush_T(j):
            # transpose of tile j's gate output, one iteration after tile j;
            # the PSUM->SBUF copy casts to fp8 for the DoubleRow zmm
            zj = z_hist[j]
            ptz_t = ptz.tile([P, 2, P], BF16, tag="ptz")
            nc.tensor.transpose(ptz_t[:, 0, :], zj[:, 0:128], idbc[:])
            nc.tensor.transpose(ptz_t[:, 1, :], zj[:, 128:256], idbc[:])
            zT = pzT.tile([P, 2, P], FP8)
            nc.vector.tensor_copy(zT[:], ptz_t[:])
            zT_hist[j] = zT

        def flush_zmm(j):
            # z @ 0.5*Wvd1 (single fp8 DoubleRow matmul) accumulated onto
            # the folded output, then drained and written out; two
            # iterations after tile j
            pdfj = pdf_hist[j]
            zT = zT_hist[j]
            mmz = nc.tensor.matmul(
                pdfj[:, 256:512], zT[:], wv18c[:],
                start=False, stop=True, skip_group_check=True, perf_mode=DR,
            )
            if j % 2 == 0:
                obufs[j // 2] = pot.tile([P, 2, E], F32, tag="obuf",
                                         name=f"obuf{j // 2}")
            ob = obufs[j // 2]
            # ACT observes the zT-copy tick so the ob drain carries a single
            # (PE) wait
            ztch = ptch.tile([1, 2], BF16, tag="tch2")
            nc.scalar.copy(ztch[:], zT[0:1, 0, 0:4].bitcast(BF16))
            ob_hist[j] = nc.scalar.copy(ob[:, j % 2, :], pdfj[:, 256:512])
            if j % 2 == 1:
                g0 = (j - 1) * P
                odst = out_d[g0 : g0 + 2 * P, :].rearrange(
                    "(t p) e -> p t e", p=P)
                nc.gpsimd.dma_start(odst, ob[:])
            return mmz

        for i in range(NTILES):
            aS = pactS.tile([P, NC_, 2, P], FP8)
            nc.sync.dma_start(aS[:], actS5[:, i, :, :, :])
            if i % 4 == 0:
                aX4 = pactX.tile([33, 4, P], BF16)
                nc.scalar.dma_start(
                    aX4[:], actd_d[:, i * P : (i + 4) * P].rearrange(
                        "p (f r) -> p f r", f=4)
                )
            aX = aX4[:, i % 4, :]

            # "PE observes processor X" gadgets: every real matmul self-loads
            # its weights, so stray LDWEIGHTS are harmless
            absorbers = [
                nc.tensor.ldweights(aS[0:1, 0, 0, 0:2]),
            ]
            if i % 4 == 0:
                absorbers.append(nc.tensor.ldweights(aX4[0:1, 0, 0:2]))
            if (i - 4) in ob_hist:
                # absorb the pdf-PSUM recycle dep (read by ob of tile i-4)
                ldw_ob = nc.tensor.ldweights(idbc[0:1, 2:4])
                add_dep_helper(ldw_ob.ins, ob_hist[i - 4].ins, sync=True,
                               reason="absorb pdf recycle ACT wait")
                absorbers.append(ldw_ob)
            if mmg_last_prev is not None:
                ldw_self = nc.tensor.ldweights(idbc[0:1, 0:2])
                add_dep_helper(ldw_self.ins, mmg_last_prev.ins, sync=True,
                               reason="absorb PE W-W completion wait")
                if mmz_prev2 is not None:
                    add_dep_helper(ldw_self.ins, mmz_prev2.ins, sync=True,
                                   reason="absorb PE W-W completion wait")
                if (i - 4) in mmdf_hist:
                    add_dep_helper(ldw_self.ins, mmdf_hist[i - 4].ins,
                                   sync=True,
                                   reason="absorb pdf WAW completion wait")
                absorbers.append(ldw_self)

            # dyn (cols 0:256) and folded-linear out part (cols 256:512),
            # one bf16 matmul
            pdf_t = pdf.tile([P, 512], F32)
            mm_df = nc.tensor.matmul(
                pdf_t[:, 0:512], aX, wdfc[:], start=True, stop=True,
            )
            mmdf_hist[i] = mm_df
            pdf_hist[i] = pdf_t
            for a in absorbers:
                add_dep_helper(mm_df.ins, a.ins, sync=False,
                               reason="absorbers run before first matmul")

            # stacked gathered-conv: 4 fp8 DoubleRow matmuls (256 contraction
            # rows each) accumulate pre = sum_k cw[t,k]*xhn_gathered directly
            pre_t = ppre.tile([P, E], F32, tag="pre")
            for c in range(NC_):
                mg = nc.tensor.matmul(
                    pre_t[:], aS[:, c, :, :], wgsc[:, c, :, :],
                    start=(c == 0), stop=(c == NC_ - 1), perf_mode=DR,
                )
                if c == 0:
                    for a in absorbers:
                        add_dep_helper(mg.ins, a.ins, sync=False,
                                       reason="absorbers first")
                mmg_last_prev = mg

            # deferred z-path: transpose of tile i-1, zmm+out of tile i-2
            if i >= 1:
                flush_T(i - 1)
            mmz_prev2 = mmz_prev
            if i >= 2:
                mmz_prev = flush_zmm(i - 2)

            # softmax chain for this tile (ACT: relu-drain + exp)
            rel = prel.tile([P, E], BF16, tag="rel")
            nc.scalar.activation(rel[:], pre_t[:], func=RELU, bias=cbc[:])
            exm = pex.tile([P, E], BF16, tag="exm")
            ssum = psml.tile([P, 1], F32, tag="sml")
            nc.scalar.activation(exm[:], rel[:], func=EXP, accum_out=ssum[:])
            sinv = psml.tile([P, 1], F32, tag="sml")
            nc.vector.reciprocal(sinv[:], ssum[:])

            # DVE observes the dyn/fold matmul once before the gate
            pdtouch = ptch.tile([1, 2], BF16, tag="tch")
            nc.vector.tensor_copy(pdtouch[:], pdf_t[:].bitcast(BF16)[0:1, 0:2])

            # z = (exm / S) * dyn  (the 0.5 is folded into Wvd1)
            z = pz.tile([P, E], BF16, tag="z")
            nc.vector.scalar_tensor_tensor(
                z[:], exm[:], sinv[:], pdf_t[:, 0:E], op0=MULT, op1=MULT
            )
            z_hist[i] = z

        flush_T(NTILES - 1)
        mmz_prev2 = mmz_prev
        mmz_prev = flush_zmm(NTILES - 2)
        flush_zmm(NTILES - 1)

    return nc


def _host_prep(x, human, nature, perm, Wv, bv, Wd, bd, Wh, bh, Wn, bn,
               conv_w, conv_b, Wvd, bvd):
    f = np.float32
    bf = ml_dtypes.bfloat16
    f8 = ml_dtypes.float8_e4m3fn
    x = np.asarray(x, f)
    human = np.asarray(human, f)
    nature = np.asarray(nature, f)
    Wv = np.asarray(Wv, f); bv = np.asarray(bv, f)
    Wd = np.asarray(Wd, f); bd = np.asarray(bd, f)
    Wh = np.asarray(Wh, f); bh = np.asarray(bh, f)
    Wn = np.asarray(Wn, f); bn = np.asarray(bn, f)
    conv_w = np.asarray(conv_w, f)
    conv_b = np.asarray(conv_b, f)
    Wvd = np.asarray(Wvd, f); bvd = np.asarray(bvd, f)
    perm = np.asarray(perm).astype(np.int64)

    Wvd1 = Wvd[:E, :]
    Wvd2 = Wvd[E:, :]
    BT = B * T

    acts = np.concatenate(
        [
            x.reshape(BT, XS),
            np.ones((BT, 1), f),
            human.reshape(BT, HT),
            nature.reshape(BT, NT_),
        ],
        axis=1,
    )
    actsT = np.ascontiguousarray(acts.T)  # [193, BT] fp32

    # stacked tap-scaled acts: S[k*193+a, r] = acts[a, r] * conv_w[t(r),0,k]
    sc = np.empty((KW, BT), f)
    for k in range(KW):
        sc[k] = np.tile(conv_w[:, 0, k], B)
    S = np.zeros((SKP, BT), f)
    for k in range(KW):
        S[k * AK : (k + 1) * AK] = actsT * sc[k][None, :]
    S8 = S.astype(f8)                                    # [1024, BT]
    actd = np.ascontiguousarray(actsT[0:33]).astype(bf)  # [33, BT]

    # dyn + folded linear path (bf16 merged matmul)
    wdf = np.zeros((33, 2 * E), f)
    wdf[0:DS, 0:E] = Wd
    wdf[32, 0:E] = bd
    wdf[0:DS, E:] = 0.5 * (Wd @ Wvd1)
    wdf[31, E:] = Wv[0] @ Wvd2
    wdf[32, E:] = 0.5 * (bd @ Wvd1) + bv @ Wvd2 + bvd
    wdf = wdf.astype(bf)

    # stacked per-tap gathered conv weights (fp8)
    ainv = np.argsort(perm)
    WgS = np.zeros((SKP, E), f)
    for k in range(KW):
        pos = ainv[:E] + k - 3
        base = k * AK
        for j in range(E):
            pj = pos[j]
            if 0 <= pj < C:
                c = perm[pj]
                if c < E:
                    WgS[base : base + DS, j] = Wd[:, c]
                    WgS[base + 32, j] = bd[c]
                elif c < 2 * E:
                    WgS[base + 33 : base + 113, j] = Wh[:, c - E]
                    WgS[base + 32, j] = bh[c - E]
                else:
                    WgS[base + 113 : base + 193, j] = Wn[:, c - 2 * E]
                    WgS[base + 32, j] = bn[c - 2 * E]
    # [ki][c][o][j] = WgS[256c+128o+ki, j]
    wgs = np.ascontiguousarray(
        WgS.reshape(NC_, 2, P, E).transpose(2, 0, 1, 3)
    ).reshape(P, NC_ * 2 * E).astype(f8)

    # [ki][o][e] = 0.5*Wvd1[128o+ki, e]
    wv18 = np.ascontiguousarray(
        (0.5 * Wvd1).reshape(2, P, E).transpose(1, 0, 2)
    ).reshape(P, 2 * E).astype(f8)

    idb = np.eye(P, dtype=bf)
    cb = np.tile(conv_b, 2).reshape(P, 1).astype(f)
    return S8, actd, wdf, wgs, wv18, idb, cb


def kernel(**inputs):
    global _NC_CACHE, LAST_RESULTS
    S8, actd, wdf, wgs, wv18, idb, cb = _host_prep(**inputs)

    if _NC_CACHE is None:
        _NC_CACHE = _build_nc()
    nc = _NC_CACHE

    in_maps = []
    for ci in range(NCORES):
        rs = slice(ci * R, (ci + 1) * R)
        # stacked rows [1024, R] -> per-tile-contiguous [p][i][c][o][128]
        sS = np.ascontiguousarray(
            S8[:, rs].reshape(NC_, 2, P, NTILES, P).transpose(2, 3, 0, 1, 4)
        ).reshape(P, NTILES * NC_ * 2 * P)
        sd = np.ascontiguousarray(actd[:, rs])
        in_maps.append({"actS": sS, "actd": sd, "wdf": wdf, "wgs": wgs,
                        "wv18": wv18, "idb": idb, "cb": cb})

    res = run_bass_kernel_spmd(nc, in_maps, core_ids=list(range(NCORES)),
                               trace=TRACE)
    LAST_RESULTS = res

    out = np.empty((B, T, E), np.float32)
    for ci in range(NCORES):
        out[ci * BPC : (ci + 1) * BPC] = res.results[ci]["out"].reshape(
            BPC, T, E)
    return out
